# revision 60
# baseline (speedup 1.0000x reference)
"""Multi-head attention layer (QKV proj + RoPE + SDPA + o_proj) on 8 TRN2 cores.

Sharding: DP2 x TP4. Core c handles batch c//4 and heads 4*(c%4)..4*(c%4)+4.
Each core computes its 4 heads' attention and a partial o_proj output
[L, D]; the host sums the 4 partials per batch (row-parallel o_proj).

All matmul operands are bf16 (same 1 cycle/row PE rate as fp32r on TRN2,
half the DMA/SBUF footprint); PSUM accumulation is fp32 throughout.

Structure (single PE stream, minimal gaps; ~437us on HW vs the 650us
fp32r baseline):
  phase 1: for each 512-token block n, stream htk once and run q, k, v
           projections back-to-back out of the same SBUF tiles. RoPE
           rot-matmuls for block n's flush are spread into the following
           projection stream so their PSUM bank reuse never stalls the PE.
           Weight prefetch is tag-major (wq first) in fine chunks so the
           first matmuls start ~4us in.
  phase 2: attention, 2-kv-tile supersteps with a 2-step software
           pipeline: scores for tiles (t, t+1) are issued before av(t-2),
           av(t-1) and a 4-matmul den batch (shared ones stationary), so
           the exp (ACT) latency and LDWEIGHTS switches hide behind PE
           work. Scores land in a [128,1024] fp32 PSUM tile (two
           single-shot 512-wide matmuls -> one 1024-wide exp; wq carries
           the 1/sqrt(Hd) scale). Softmax skips max-subtraction (scores
           ~N(0,1)). Tail per (head, half): DVE copy-out, fast approx
           reciprocal, DRAM-bounce partition broadcast, in-place DVE
           normalize.
  phase 3: o_proj per token tile with all 4 d-blocks accumulating at
           once (each outT stationary serves 4 matmuls, 8 PSUM banks),
           PSUM->SBUF copies alternating DVE/ACT, bf16 output DMA
           (summed to fp32 on the host).

Accumulation-chain rule learned the hard way: `start=True` clears the
has_written bits for the WHOLE PSUM bank, so two interleaved multi-step
accumulation chains must never share a bank (single-shot matmuls may).
"""

import numpy as np

import sys
import types

# Defensive: concourse.bass_utils imports antenv.axon_hooks when tracing is
# requested; provide a null shim if the module is absent in this image so a
# stray BASS_TRACE env var cannot crash the kernel.
try:
    import antenv.axon_hooks  # noqa: F401
except ImportError:
    _m = types.ModuleType("antenv.axon_hooks")
    _m.set_axon_ntff_profile_hook = lambda h: None
    _m.get_axon_ntff_profile_hook = lambda: None
    sys.modules["antenv.axon_hooks"] = _m

import ml_dtypes

import concourse.bass as bass
import concourse.mybir as mybir
import concourse.tile as tile
from concourse import bacc
from concourse.bass_utils import run_bass_kernel_spmd

# problem constants (hardcoded per spec)
B, L, D = 2, 2048, 2048
H, Hd = 16, 128
NC = 8
TPH = 4            # heads per core
QKV = TPH * Hd     # 512 per-core projection width
KT = D // 128      # 16 contraction tiles
NT = L // 512      # 4 token groups of 512
MT = L // 128      # 16 token chunks of 128

f32 = mybir.dt.float32
bf16 = mybir.dt.bfloat16

AF = mybir.ActivationFunctionType
SCALE = 1.0 / float(np.sqrt(Hd))

_CACHE: dict = {}


def _build():
    nc = bacc.Bacc("TRN2", target_bir_lowering=False, debug=False)

    # inputs are pre-tiled on the host so every DMA line is contiguous per
    # partition (4-16KB instead of 1KB)
    hTt = nc.dram_tensor("hTt", [NT, 128, KT, 512], bf16, kind="ExternalInput").ap()
    wqT = nc.dram_tensor("wqT", [128, KT, QKV], bf16, kind="ExternalInput").ap()
    wkT = nc.dram_tensor("wkT", [128, KT, QKV], bf16, kind="ExternalInput").ap()
    wvT = nc.dram_tensor("wvT", [128, KT, QKV], bf16, kind="ExternalInput").ap()
    woT = nc.dram_tensor("woT", [128, TPH, D], bf16, kind="ExternalInput").ap()
    cosT = nc.dram_tensor("cosT", [Hd, L], bf16, kind="ExternalInput").ap()
    sinTs = nc.dram_tensor("sinTs", [Hd, L], bf16, kind="ExternalInput").ap()
    rotM = nc.dram_tensor("rotM", [Hd, Hd], bf16, kind="ExternalInput").ap()
    out = nc.dram_tensor("out", [L, D], bf16, kind="ExternalOutput").ap()

    out_re = out.rearrange("(mm p) (nb d) -> p mm nb d", p=128, d=512)

    with tile.TileContext(nc) as tc:
        with tc.tile_pool(name="persist", bufs=1) as persist:
            # ---- persistent tensors -----------------------------------
            ones_b = persist.tile([128, 1], bf16, name="ones_b")
            nc.vector.memset(ones_b, 1.0)
            warm = persist.tile([128, 512], bf16, name="warm")
            nc.vector.memset(warm, 0.0)
            qT = [persist.tile([Hd, L], bf16, name=f"qT{h}") for h in range(TPH)]
            kT = [persist.tile([Hd, L], bf16, name=f"kT{h}") for h in range(TPH)]
            v_big = persist.tile([128, MT, QKV], bf16, name="v_big")
            outT = [persist.tile([Hd, L], bf16, name=f"outT{h}") for h in range(TPH)]
            cos_sb = persist.tile([Hd, L], bf16, name="cos_sb")
            sin_sb = persist.tile([Hd, L], bf16, name="sin_sb")
            rot_sb = persist.tile([Hd, Hd], bf16, name="rot_sb")

            # ==== phase 1: fused q/k/v projections + RoPE ====================
            with (
                tc.tile_pool(name="wqkv", bufs=1) as wqkv,
                tc.tile_pool(name="stream", bufs=2) as stream,
                tc.tile_pool(name="tmp", bufs=2) as tmp,
                tc.tile_pool(name="pp", bufs=1, space="PSUM") as pp,
                tc.tile_pool(name="psr", bufs=2, space="PSUM") as psr,
            ):
                # first htk block ahead of everything else on the sync queue
                htk0 = stream.tile([128, KT, 512], bf16, name="htk")
                for kg in range(4):
                    nc.sync.dma_start(
                        out=htk0[:, kg * 4 : (kg + 1) * 4, :],
                        in_=hTt[0, :, kg * 4 : (kg + 1) * 4, :],
                    )
                # weight prefetch, tag-major: q is consumed first (+4us),
                # then k (+17us), then v (+31us)
                w_sbs = {}
                for tag in ("q", "k", "v"):
                    w_sbs[tag] = wqkv.tile([128, KT, QKV], bf16, name=f"w_{tag}")
                w_res = {"q": wqT, "k": wkT, "v": wvT}
                for tag in ("q", "k", "v"):
                    w_re = w_res[tag]
                    for a, b in (
                        (0, 1), (1, 2), (2, 4), (4, 6), (6, 8),
                        (8, 10), (10, 12), (12, 16),
                    ):
                        nc.gpsimd.dma_start(
                            out=w_sbs[tag][:, a:b, :], in_=w_re[:, a:b, :]
                        )
                # rope constants: needed at the first flush (~14us in)
                nc.sync.dma_start(out=cos_sb, in_=cosT)
                nc.sync.dma_start(out=sin_sb, in_=sinTs)
                nc.sync.dma_start(out=rot_sb, in_=rotM)

                # PE warm-up: junk matmuls during the initial DMA wait so the
                # clock ramp (free-running ~3.4us activity window) completes
                # before real data lands
                ps_warm = pp.tile([128, 512], f32, name="pp0", bufs=2)
                for _ in range(7):
                    nc.tensor.matmul(
                        ps_warm, warm[:, 0:128], warm, start=True, stop=True
                    )

                # deferred rope work: list of closures, emitted one per kg
                # boundary of the *following* projection stream so the psr
                # bank reuse (rot matmul -> DVE reads) never stalls the PE.
                pending_rope = []

                def emit_one_rope():
                    if pending_rope:
                        pending_rope.pop(0)()

                def make_rope(dst, m, n, raw):
                    csl = slice(n * 512, (n + 1) * 512)

                    def do():
                        ps_rot = psr.tile([128, 512], f32, name="ps_rot")
                        nc.tensor.matmul(ps_rot, rot_sb, raw, start=True, stop=True)
                        t1 = tmp.tile([128, 512], f32, name="t1")
                        nc.vector.tensor_mul(t1, raw, cos_sb[:, csl])
                        t2 = tmp.tile([128, 512], f32, name="t2")
                        nc.vector.tensor_mul(t2, ps_rot, sin_sb[:, csl])
                        nc.vector.tensor_add(dst[m][:, csl], t1, t2)

                    return do

                for n in range(NT):
                    if n == 0:
                        htk = htk0
                    else:
                        htk = stream.tile([128, KT, 512], bf16, name="htk")
                        for kg in range(2):
                            nc.sync.dma_start(
                                out=htk[:, kg * 8 : (kg + 1) * 8, :],
                                in_=hTt[n, :, kg * 8 : (kg + 1) * 8, :],
                            )
                    for tag, dst in (("q", qT), ("k", kT), ("v", None)):
                        w_sb = w_sbs[tag]
                        ps_x = [
                            pp.tile(
                                [128, 512], f32, name=f"pp{m}",
                                bufs=2 if m < 2 else 1,
                            )
                            for m in range(TPH)
                        ]
                        if tag == "v":
                            # v: stationary = token columns of htk, moving = wv
                            # rows; out [token128, qkv512] per token sub-tile.
                            for kg in range(4):
                                for mc in range(4):
                                    for i in range(4):
                                        kk = kg * 4 + i
                                        nc.tensor.matmul(
                                            ps_x[mc],
                                            htk[:, kk, mc * 128 : (mc + 1) * 128],
                                            w_sb[:, kk, :],
                                            start=(kk == 0),
                                            stop=(kk == KT - 1),
                                        )
                                emit_one_rope()
                            for mc in range(4):
                                nc.scalar.copy(v_big[:, n * 4 + mc, :], ps_x[mc])
                        else:
                            # q/k: stationary = weight m-tile, moving = htk.
                            # m-major within each kg so the first matmul of
                            # m2/m3 (single-buffered banks) comes late enough
                            # for the previous flush to have freed them.
                            for kg in range(4):
                                for m in range(TPH):
                                    for i in range(4):
                                        kk = kg * 4 + i
                                        nc.tensor.matmul(
                                            ps_x[m],
                                            w_sb[:, kk, m * 128 : (m + 1) * 128],
                                            htk[:, kk, :],
                                            start=(kk == 0),
                                            stop=(kk == KT - 1),
                                        )
                                emit_one_rope()
                            # flush: raw copies split ACT/DVE, rope deferred
                            raws = []
                            for m in range(TPH):
                                raw = tmp.tile([128, 512], bf16, name="raw", bufs=4)
                                if m < 2:
                                    nc.scalar.copy(raw, ps_x[m])
                                else:
                                    nc.vector.tensor_copy(raw, ps_x[m])
                                raws.append(raw)
                            for m in range(TPH):
                                pending_rope.append(make_rope(dst, m, n, raws[m]))
                while pending_rope:
                    emit_one_rope()

            # ==== phase 2: attention + o_proj ===============================
            with (
                tc.tile_pool(name="wo", bufs=1) as wop,
                tc.tile_pool(name="att", bufs=2) as att,
                tc.tile_pool(name="dramp", bufs=2, space="DRAM") as dramp,
            ):
                wo_sb = wop.tile([128, TPH, D], bf16, name="wo_sb")
                for hh in range(TPH):
                    nc.gpsimd.dma_start(out=wo_sb[:, hh, :], in_=woT[:, hh, :])

                HW = 1024  # tq half-width

                oproj_n = [0]

                seq = [
                    (half, h, tk)
                    for half in range(2)
                    for h in range(TPH)
                    for tk in range(MT)
                ]
                n_seq = len(seq)
                state = {}  # (half,h) -> (ps_out, den)
                probs_by_idx = {}
                partials = {}

                PIPE = 2  # av/den trail scores by 2 steps so exp() is done
                with (
                    tc.tile_pool(name="pss", bufs=2, space="PSUM") as pss,
                    tc.tile_pool(name="pso", bufs=1, space="PSUM") as pso,
                    tc.tile_pool(name="psd", bufs=1, space="PSUM") as psd,
                ):
                    def front(t):
                        half, h, tk = seq[t]
                        # full-width scores tile (2 banks); each 512-wide
                        # matmul is single-shot (start&stop) into its own
                        # bank. One 1024-wide exp serves both. wq is
                        # pre-scaled by 1/sqrt(Hd) on the host.
                        sc_ps = pss.tile([128, HW], f32, name="sc")
                        for j in range(2):
                            tq0 = half * HW + j * 512
                            nc.tensor.matmul(
                                sc_ps[:, j * 512 : (j + 1) * 512],
                                kT[h][:, tk * 128 : (tk + 1) * 128],
                                qT[h][:, tq0 : tq0 + 512],
                                start=True,
                                stop=True,
                            )
                        probs = att.tile([128, HW], bf16, name="probs", bufs=5)
                        probs_by_idx[t] = probs
                        nc.scalar.activation(probs, sc_ps, AF.Exp)

                    def back_av(t):
                        half, h, tk = seq[t]
                        if (half, h) not in state:
                            ps_out = pso.tile([Hd, HW], f32, name="ps_out")
                            den_ps = psd.tile([1, HW], f32, name="den")
                            state[(half, h)] = (ps_out, den_ps)
                        ps_out, den_ps = state[(half, h)]
                        probs = probs_by_idx.pop(t)
                        st = dict(start=(tk == 0), stop=(tk == MT - 1))
                        for j in range(2):
                            nc.tensor.matmul(
                                ps_out[:, j * 512 : (j + 1) * 512],
                                v_big[:, tk, h * 128 : (h + 1) * 128],
                                probs[:, j * 512 : (j + 1) * 512],
                                **st,
                            )
                        # denominator: accumulate probs tiles on the (idle)
                        # DVE into 4 bf16 partial sums per unit; the PE then
                        # only reduces the partials (8 matmuls/unit instead
                        # of 32). bf16 partial rounding adds ~0.1% den error.
                        p_idx = tk // 4
                        parts = partials.setdefault((half, h), [None] * 4)
                        # even partials on DVE, odd on the idle GPSIMD so
                        # neither engine's accumulation chain limits the PE
                        eng = nc.vector if p_idx % 2 == 0 else nc.gpsimd
                        if tk % 4 == 0:
                            pt = att.tile(
                                [128, HW], bf16, name=f"part{p_idx}", bufs=2
                            )
                            parts[p_idx] = pt
                            eng.tensor_copy(pt, probs)
                        else:
                            pt = parts[p_idx]
                            eng.tensor_add(pt, pt, probs)

                    def tail(t):
                        half, h, tk = seq[t]
                        ps_out, den_ps = state.pop((half, h))
                        # reduce the 4 partial sums into the denominator row
                        parts = partials.pop((half, h))
                        for i in range(4):
                            for j in range(2):
                                nc.tensor.matmul(
                                    den_ps[0:1, j * 512 : (j + 1) * 512],
                                    ones_b,
                                    parts[i][:, j * 512 : (j + 1) * 512],
                                    start=(i == 0),
                                    stop=(i == 3),
                                )
                        # rest off the PE: copy out unnormalized, fast
                        # reciprocal, DRAM-bounce broadcast, in-place
                        # normalize.
                        sl = slice(half * HW, (half + 1) * HW)
                        nc.vector.tensor_copy(outT[h][:, sl], ps_out)
                        rec = att.tile([1, HW], f32, name="rec", bufs=2)
                        nc.vector.reciprocal_approx_fast(rec, den_ps[0:1, :])
                        rec_dram = dramp.tile([1, HW], f32, name="rec_dram")
                        nc.gpsimd.dma_start(out=rec_dram, in_=rec)
                        rec_bc = att.tile([128, HW], f32, name="rec_bc", bufs=2)
                        bc_ap = bass.AP(
                            tensor=rec_dram.tensor,
                            offset=rec_dram.offset,
                            ap=[[0, 128]] + [list(x) for x in rec_dram.ap[1:]],
                        )
                        nc.gpsimd.dma_start(out=rec_bc, in_=bc_ap)
                        nc.vector.tensor_mul(
                            outT[h][:, sl], outT[h][:, sl], rec_bc
                        )

                    # 2-wide supersteps: sc pairs for (s, s+1) then av pairs
                    # for (s-2, s-1) then the batched dens; keeps every
                    # LDWEIGHTS under a long same-weights stream.
                    for s in range(0, n_seq + PIPE, 2):
                        for t in (s, s + 1):
                            if t < n_seq:
                                front(t)
                        for t in (s - PIPE, s - PIPE + 1):
                            if 0 <= t < n_seq:
                                back_av(t)
                        for t in (s - PIPE, s - PIPE + 1):
                            if 0 <= t < n_seq and seq[t][2] == MT - 1:
                                tail(t)
                # o_proj (all token tiles); attention PSUM banks are free
                # now: accumulate all 4 d-blocks of a token tile at once so
                # each outT stationary load serves 4 matmuls.
                with tc.tile_pool(name="psf2", bufs=1, space="PSUM") as psf2:
                    for m in range(16):
                        ps_f4 = [
                            psf2.tile([128, 512], f32, name=f"ps_f4_{nb}", bufs=2)
                            for nb in range(4)
                        ]
                        for hh in range(TPH):
                            for nblk in range(4):
                                nc.tensor.matmul(
                                    ps_f4[nblk],
                                    outT[hh][:, m * 128 : (m + 1) * 128],
                                    wo_sb[:, hh, nblk * 512 : (nblk + 1) * 512],
                                    start=(hh == 0),
                                    stop=(hh == TPH - 1),
                                )
                        # stage all 4 d-blocks, then write the token tile's
                        # full 2048-col row in one DMA (4KB per partition)
                        ot4 = att.tile([128, 4, 512], bf16, name="ot4", bufs=3)
                        for nblk in range(4):
                            if (oproj_n[0] + nblk) % 2 == 0:
                                nc.vector.tensor_copy(
                                    ot4[:, nblk, :], ps_f4[nblk]
                                )
                            else:
                                nc.scalar.copy(ot4[:, nblk, :], ps_f4[nblk])
                        oproj_n[0] += 1
                        nc.sync.dma_start(out=out_re[:, m, :, :], in_=ot4)

    nc.compile()
    return nc


def _bf(x: np.ndarray) -> np.ndarray:
    return np.ascontiguousarray(x, dtype=np.float32).astype(ml_dtypes.bfloat16)


def kernel(hidden_states, cos, sin, wq, wk, wv, wo):
    if "nc" not in _CACHE:
        _CACHE["nc"] = _build()
    nc = _CACHE["nc"]

    hidden_states = np.asarray(hidden_states, dtype=np.float32)
    cos = np.asarray(cos, dtype=np.float32)
    sin = np.asarray(sin, dtype=np.float32)
    wq = np.asarray(wq, dtype=np.float32)
    wk = np.asarray(wk, dtype=np.float32)
    wv = np.asarray(wv, dtype=np.float32)
    wo = np.asarray(wo, dtype=np.float32)

    # host-side layout prep
    cosT = _bf(cos[0, 0].T)                             # [Hd, L]
    sinT = np.ascontiguousarray(sin[0, 0].T)            # [Hd, L]
    sinTs = sinT.copy()
    sinTs[: Hd // 2] *= -1.0                            # fold rotate_half signs
    sinTs = _bf(sinTs)
    rot = np.zeros((Hd, Hd), dtype=np.float32)          # pure half-swap permutation
    for p in range(Hd // 2):
        rot[p, p + Hd // 2] = 1.0
        rot[p + Hd // 2, p] = 1.0
    rotM = _bf(rot.T)

    # pre-tile for contiguous per-partition DMA lines:
    #   hTt[n, p, kk, t] = h.T[kk*128+p, n*512+t]
    #   w*T[p, kk, r]    = w[r, kk*128+p]   (w.T row d = kk*128+p)
    #   woT[p, hh, d]    = wo[d, r0+hh*128+p]
    def _tile_h(hb):
        return _bf(
            hb.T.reshape(KT, 128, NT, 512).transpose(2, 1, 0, 3)
        )

    def _tile_w(wrows):
        return _bf(wrows.T.reshape(KT, 128, QKV).transpose(1, 0, 2))

    def _tile_wo(wcols):
        return _bf(wcols.T.reshape(TPH, 128, D).transpose(1, 0, 2))

    hTt = [_tile_h(hidden_states[b]) for b in range(B)]
    # fold the attention scale into wq so raw scores are ~N(0,1) (keeps the
    # bf16 PSUM score rounding relative) and exp() needs no scale
    wq = wq * SCALE

    in_maps = []
    for c in range(NC):
        b = c // 4
        hb = c % 4
        r0 = hb * QKV
        in_maps.append(
            {
                "hTt": hTt[b],
                "wqT": _tile_w(wq[r0 : r0 + QKV]),
                "wkT": _tile_w(wk[r0 : r0 + QKV]),
                "wvT": _tile_w(wv[r0 : r0 + QKV]),
                "woT": _tile_wo(wo[:, r0 : r0 + QKV]),
                "cosT": cosT,
                "sinTs": sinTs,
                "rotM": rotM,
            }
        )

    res = run_bass_kernel_spmd(nc, in_maps, core_ids=list(range(NC)))
    _CACHE["last_results"] = res

    out = np.zeros((B, L, D), dtype=np.float32)
    for c in range(NC):
        out[c // 4] += np.asarray(res.results[c]["out"], dtype=np.float32)
    return out
